# revision 14
# baseline (speedup 1.0000x reference)
"""Causal self-attention (B=4, T=2048, C=768, H=12, RoPE) on 8 TRN2 NeuronCores.

Sharding: core c -> (batch b = c//2, head-group g = c%2 of 6 heads).
Each core computes, for its batch element and its 6 heads:
    qkv^T-style projections, RoPE, causal attention, and the partial output
    projection  attn_out @ W_proj[rows of its heads].
Host sums the two partial outputs per batch and adds the (host-folded)
bias terms:  out[b] = part[2b] + part[2b+1] + b_proj + b_v @ W_proj.

On-chip layout (per core):
  xt   [C, T]   = x[b].T                       (f32r, matmul operand)
  wqk  [C, 768] = [Wq_g | Wk_g]                (f32r)
  wv   [C, 384] = Wv_g                         (f32r)
  wp   [384, C] = W_proj rows of group         (f32r)
  Q^T/K^T built as [128, T] "m-tiles" (2 heads each) via
  out = wqk_mtile.T @ xt  with RoPE applied by
  y = raw*CC + rowswap(raw)*SS  (rowswap via SB->SB DMA partition permute).
  Scores are computed transposed: S^T[k, q] chunks [128, 512], exp on ACT,
  causal mask via affine_select, then Y'^T = [V|1]^T-chain accumulation
  giving both Y^T (rows 0-63) and softmax denominators (row 64).
"""
import sys
sys.path.insert(0, "/opt/trn_rl_repo")

import numpy as np

ROPE_BASE = 10000.0
NCORES = 8

_CACHE = {}


def _rope_tables(T):
    inv_freq = 1.0 / (ROPE_BASE ** (np.arange(0, 64, 2, dtype=np.float64) / 64))
    t = np.arange(T, dtype=np.float64)
    fr = np.outer(t, inv_freq)            # [T, 32]
    cosT = np.cos(fr).T.astype(np.float32)   # [32, T]
    sinT = np.sin(fr).T.astype(np.float32)
    CC = np.tile(cosT, (4, 1))            # [128, T]
    SS = np.concatenate([sinT, -sinT, sinT, -sinT], axis=0)  # [128, T]
    return CC, SS


def build_nc(C, T, HPC, debug=False):
    """Build the per-core Bass program. C: contraction dim, T: seq len,
    HPC: heads per core (even)."""
    import concourse.bass as bass
    import concourse.tile as tile
    from concourse import bacc, mybir

    F32 = mybir.dt.float32
    F32R = mybir.dt.float32r
    Act = mybir.ActivationFunctionType

    KT = C // 128          # contraction k-tiles
    NP = HPC // 2          # head pairs
    MT = 2 * NP            # qk m-tiles (Q tiles then K tiles)
    TT = T // 128          # 128-row t-tiles (= key chunks)
    QC = T // 512          # query chunks of 512
    VC = 64 * HPC          # v columns
    PC = VC // 128         # projection contraction k-tiles (= NP)

    nc = bacc.Bacc("TRN2", target_bir_lowering=False, debug=False)

    xt_d = nc.dram_tensor("xt", [C, T], F32R, kind="ExternalInput")
    wqk_d = nc.dram_tensor("wqk", [C, 2 * VC], F32R, kind="ExternalInput")
    bqk_d = nc.dram_tensor("bqk", [2 * VC], F32, kind="ExternalInput")
    wv_d = nc.dram_tensor("wv", [C, VC], F32R, kind="ExternalInput")
    wp_d = nc.dram_tensor("wp", [VC, C], F32R, kind="ExternalInput")
    out_d = nc.dram_tensor("out", [T, C], F32, kind="ExternalOutput")
    if debug:
        KTl = C // 128; NPl = HPC // 2; MTl = HPC; TTl = T // 128; QCl = T // 512
        dbg_qk = nc.dram_tensor("dbg_qk", [MTl, 128, T], F32, kind="ExternalOutput")
        dbg_vp = nc.dram_tensor("dbg_vp", [128, TTl * HPC * 65], F32, kind="ExternalOutput")
        dbg_yt = nc.dram_tensor("dbg_yt", [NPl, 128, T], F32, kind="ExternalOutput")
        dbg_d = nc.dram_tensor("dbg_d", [NPl * QCl, 2, 512], F32, kind="ExternalOutput")
        dbg_bc = nc.dram_tensor("dbg_bc", [NPl * QCl, 2, 64, 512], F32, kind="ExternalOutput")

    rsc_d = nc.dram_tensor("rscratch", [HPC // 2, T // 512, 2, 512], F32R)

    CCh, SSh = _rope_tables(T)
    cc_d = nc.inline_tensor(CCh, name="rope_cc")
    ss_d = nc.inline_tensor(SSh, name="rope_ss")

    with nc.allow_low_precision(reason="fp32r matmul pipeline"), \
         tile.TileContext(nc) as tc:
        import contextlib
        with contextlib.ExitStack() as ctx:
            # ---- long-lived pools -------------------------------------
            big = ctx.enter_context(tc.tile_pool(name="big", bufs=1))
            qk_pool = ctx.enter_context(tc.tile_pool(name="qks", bufs=1))
            vp_pool = ctx.enter_context(tc.tile_pool(name="vp", bufs=1))
            misc = ctx.enter_context(tc.tile_pool(name="misc", bufs=1))

            # xt and Yt share one slot (same tag, sequential lifetimes)
            xt_sb = big.tile([128, KT, T], F32R, tag="bigshare")

            qks = [qk_pool.tile([128, T], F32R, tag=f"qk{m}", name=f"qk{m}")
                   for m in range(MT)]
            vp_sb = vp_pool.tile([128, TT, HPC, 65], F32R)
            bqk_sb = misc.tile([128, MT], F32)
            nc.vector.tensor_copy(
                vp_sb[:, :, :, 64:65],
                nc.const_aps.tensor(1.0, (128, TT, HPC, 1)))

            for k in range(KT):
                nc.sync.dma_start(xt_sb[:, k, :], xt_d.ap()[k * 128:(k + 1) * 128, :])
            nc.sync.dma_start(
                bqk_sb[:],
                bqk_d.ap().rearrange("(m p) -> p m", p=128))

            # ---- stage B: projections + RoPE --------------------------
            with tc.tile_pool(name="wqk", bufs=1) as wqk_pool, \
                 tc.tile_pool(name="wv", bufs=1) as wv_pool, \
                 tc.tile_pool(name="ccss", bufs=1) as ccss_pool, \
                 tc.tile_pool(name="raw", bufs=3) as raw_pool, \
                 tc.tile_pool(name="swp", bufs=3) as swp_pool, \
                 tc.tile_pool(name="qkps", bufs=3, space="PSUM") as qkps, \
                 tc.tile_pool(name="vps", bufs=2, space="PSUM") as vps:

                wqk_sb = wqk_pool.tile([128, KT, 2 * VC], F32R)
                wv_sb = wv_pool.tile([128, KT, VC], F32R)
                cc_sb = ccss_pool.tile([128, T], F32)
                ss_sb = ccss_pool.tile([128, T], F32)
                for k in range(KT):
                    nc.sync.dma_start(wqk_sb[:, k, :],
                                      wqk_d.ap()[k * 128:(k + 1) * 128, :])
                    nc.sync.dma_start(wv_sb[:, k, :],
                                      wv_d.ap()[k * 128:(k + 1) * 128, :])
                nc.sync.dma_start(cc_sb, cc_d.ap())
                nc.sync.dma_start(ss_sb, ss_d.ap())

                # V projection: v = x @ wv  ([T,VC]); store per key-chunk
                for tt in range(TT):
                    vpsum = vps.tile([128, VC], F32)
                    for k in range(KT):
                        nc.tensor.matmul(
                            vpsum, xt_sb[:, k, tt * 128:(tt + 1) * 128],
                            wv_sb[:, k, :], start=(k == 0), stop=(k == KT - 1))
                    nc.vector.tensor_copy(
                        vp_sb[:, tt, :, 0:64],
                        vpsum[:].rearrange("p (h d) -> p h d", h=HPC))

                # Q^T / K^T m-tiles with RoPE; order pairs for early stage-C
                m_order = []
                for j in range(NP):
                    m_order += [j, NP + j]
                for m in m_order:
                    for ch in range(T // 512):
                        psum = qkps.tile([128, 512], F32)
                        for k in range(KT):
                            nc.tensor.matmul(
                                psum, wqk_sb[:, k, m * 128:(m + 1) * 128],
                                xt_sb[:, k, ch * 512:(ch + 1) * 512],
                                start=(k == 0), stop=(k == KT - 1))
                        raw = raw_pool.tile([128, 512], F32)
                        nc.vector.tensor_scalar_add(raw, psum, bqk_sb[:, m:m + 1])
                        swp = swp_pool.tile([128, 512], F32)
                        for h in (0, 64):
                            nc.sync.dma_start(swp[h:h + 32, :], raw[h + 32:h + 64, :])
                            nc.sync.dma_start(swp[h + 32:h + 64, :], raw[h:h + 32, :])
                        nc.vector.tensor_mul(raw, raw, cc_sb[:, ch * 512:(ch + 1) * 512])
                        nc.vector.tensor_mul(swp, swp, ss_sb[:, ch * 512:(ch + 1) * 512])
                        nc.vector.tensor_add(
                            qks[m][:, ch * 512:(ch + 1) * 512], raw, swp)

            if debug:
                for m in range(MT):
                    nc.sync.dma_start(dbg_qk.ap()[m], qks[m][:].bitcast(F32))
                nc.sync.dma_start(
                    dbg_vp.ap(),
                    vp_sb[:].rearrange("p a b c -> p (a b c)").bitcast(F32))

            # ---- stage C: attention -----------------------------------
            yt_sb = big.tile([128, NP, T], F32R, tag="bigshare")
            with tc.tile_pool(name="pt", bufs=4) as pt_pool, \
                 tc.tile_pool(name="ra", bufs=2) as ra_pool, \
                 tc.tile_pool(name="tb", bufs=2) as tb_pool, \
                 tc.tile_pool(name="sps", bufs=3, space="PSUM") as sps, \
                 tc.tile_pool(name="yps", bufs=1, space="PSUM") as yps, \
                 tc.tile_pool(name="bcs", bufs=2) as bcs_pool:
                for j in range(NP):
                    qt, kt = qks[j], qks[NP + j]
                    for qc in range(QC):
                        nkc = 4 * (qc + 1)
                        ya = yps.tile([65, 512], F32, tag="ya")
                        yb = yps.tile([65, 512], F32, tag="yb")
                        for kc in range(nkc):
                            qs = slice(qc * 512, (qc + 1) * 512)
                            ks = slice(kc * 128, (kc + 1) * 128)
                            sa = sps.tile([128, 512], F32, tag="sa")
                            sb_ = sps.tile([128, 512], F32, tag="sb")
                            nc.tensor.matmul(sa, kt[0:64, ks], qt[0:64, qs],
                                             start=True, stop=True)
                            nc.tensor.matmul(sb_, kt[64:128, ks], qt[64:128, qs],
                                             start=True, stop=True)
                            pa = pt_pool.tile([128, 512], F32R, tag="pa")
                            pb = pt_pool.tile([128, 512], F32R, tag="pb")
                            nc.scalar.activation(pa, sa, Act.Exp, scale=0.125)
                            nc.scalar.activation(pb, sb_, Act.Exp, scale=0.125)
                            if kc >= 4 * qc:  # diagonal: mask k > q -> 0
                                for p in (pa, pb):
                                    nc.gpsimd.affine_select(
                                        out=p, in_=p,
                                        compare_op=mybir.AluOpType.is_ge,
                                        fill=0.0,
                                        base=qc * 512 - kc * 128,
                                        channel_multiplier=-1,
                                        pattern=[[1, 512]])
                            nc.tensor.matmul(ya, vp_sb[:, kc, 2 * j, :], pa,
                                             start=(kc == 0), stop=(kc == nkc - 1))
                            nc.tensor.matmul(yb, vp_sb[:, kc, 2 * j + 1, :], pb,
                                             start=(kc == 0), stop=(kc == nkc - 1))
                        qs = slice(qc * 512, (qc + 1) * 512)
                        ra = ra_pool.tile([65, 512], F32R, tag="ra")
                        rb = ra_pool.tile([65, 512], F32R, tag="rb")
                        nc.vector.reciprocal(ra[64:65, :], ya[64:65, :])
                        nc.vector.reciprocal(rb[64:65, :], yb[64:65, :])
                        bca = bcs_pool.tile([64, 512], F32R, tag="bca")
                        bcb = bcs_pool.tile([64, 512], F32R, tag="bcb")
                        nc.sync.dma_start(rsc_d.ap()[j, qc, 0], ra[64:65, :])
                        nc.sync.dma_start(rsc_d.ap()[j, qc, 1], rb[64:65, :])
                        nc.sync.dma_start(
                            bca, rsc_d.ap()[j, qc, 0:1, :].to_broadcast((64, 512)))
                        nc.sync.dma_start(
                            bcb, rsc_d.ap()[j, qc, 1:2, :].to_broadcast((64, 512)))
                        nc.vector.tensor_mul(yt_sb[0:64, j, qs], ya[0:64, :], bca)
                        tb = tb_pool.tile([64, 512], F32R)
                        nc.vector.tensor_mul(tb, yb[0:64, :], bcb)
                        nc.sync.dma_start(yt_sb[64:128, j, qs], tb)
                        if debug:
                            nc.sync.dma_start(dbg_d.ap()[j * QC + qc, 0], ra[64:65, :].bitcast(F32))
                            nc.sync.dma_start(dbg_d.ap()[j * QC + qc, 1], rb[64:65, :].bitcast(F32))
                            nc.sync.dma_start(dbg_bc.ap()[j * QC + qc, 0], bca[:].bitcast(F32))
                            nc.sync.dma_start(dbg_bc.ap()[j * QC + qc, 1], bcb[:].bitcast(F32))

            if debug:
                for j in range(NP):
                    nc.sync.dma_start(dbg_yt.ap()[j], yt_sb[:, j, :].bitcast(F32))

            # ---- stage D: output projection ---------------------------
            with tc.tile_pool(name="wp", bufs=1) as wp_pool, \
                 tc.tile_pool(name="osb", bufs=3) as osb_pool, \
                 tc.tile_pool(name="pps", bufs=3, space="PSUM") as pps:
                wp_sb = wp_pool.tile([128, PC, C], F32R)
                for k in range(PC):
                    nc.sync.dma_start(wp_sb[:, k, :],
                                      wp_d.ap()[k * 128:(k + 1) * 128, :])
                ccw = 384 if C % 384 == 0 else C  # proj column chunk width
                ncc = C // ccw
                for tt in range(TT):
                    osb = osb_pool.tile([128, C], F32)
                    for cc in range(ncc):
                        cs = slice(cc * ccw, (cc + 1) * ccw)
                        psum = pps.tile([128, ccw], F32)
                        for k in range(PC):
                            nc.tensor.matmul(
                                psum, yt_sb[:, k, tt * 128:(tt + 1) * 128],
                                wp_sb[:, k, cs],
                                start=(k == 0), stop=(k == PC - 1))
                        nc.vector.tensor_copy(osb[:, cs], psum)
                    nc.sync.dma_start(out_d.ap()[tt * 128:(tt + 1) * 128, :], osb)

    nc.compile()
    return nc


def _run(nc, in_maps):
    from concourse.bass_utils import run_bass_kernel_spmd
    return run_bass_kernel_spmd(nc, in_maps, core_ids=list(range(len(in_maps))))


def kernel(**inputs):
    x = np.ascontiguousarray(np.asarray(inputs["x"], dtype=np.float32))
    W = np.asarray(inputs["W_attn"], dtype=np.float32)
    b = np.asarray(inputs["b_attn"], dtype=np.float32)
    Wp = np.asarray(inputs["W_proj"], dtype=np.float32)
    bp = np.asarray(inputs["b_proj"], dtype=np.float32)
    B, T, C = x.shape

    if "nc" not in _CACHE:
        _CACHE["nc"] = build_nc(C, T, 6)
    nc = _CACHE["nc"]

    in_maps = []
    for c in range(NCORES):
        bb, g = divmod(c, 2)
        s = 384 * g
        in_maps.append({
            "xt": np.ascontiguousarray(x[bb].T),
            "wqk": np.ascontiguousarray(
                np.concatenate([W[:, s:s + 384], W[:, 768 + s:768 + s + 384]],
                               axis=1)),
            "bqk": np.ascontiguousarray(
                np.concatenate([b[s:s + 384], b[768 + s:768 + s + 384]])),
            "wv": np.ascontiguousarray(W[:, 1536 + s:1536 + s + 384]),
            "wp": np.ascontiguousarray(Wp[s:s + 384, :]),
        })

    res = _run(nc, in_maps).results
    extra = (bp + b[1536:2304] @ Wp).astype(np.float32)  # [C]
    out = np.empty((B, T, C), dtype=np.float32)
    for bb in range(B):
        out[bb] = res[2 * bb]["out"] + res[2 * bb + 1]["out"] + extra
    return out


# revision 25
# speedup vs baseline: 1.3706x; 1.3706x over previous
"""Causal self-attention (B=4, T=2048, C=768, H=12, RoPE) on 8 TRN2 NeuronCores.

Sharding: core c -> (batch b = c//2, head-group g = c%2 of 6 heads).
Each core computes, for its batch element and its 6 heads:
    qkv^T-style projections, RoPE, causal attention, and the partial output
    projection  attn_out @ W_proj[rows of its heads].
Host sums the two partial outputs per batch and adds the (host-folded)
bias terms:  out[b] = part[2b] + part[2b+1] + b_proj + b_v @ W_proj.

On-chip layout (per core):
  xt   [C, T]   = x[b].T                       (f32r, matmul operand)
  wqk  [C, 768] = [Wq_g | Wk_g]                (f32r)
  wv   [C, 384] = Wv_g                         (f32r)
  wp   [384, C] = W_proj rows of group         (f32r)
  Q^T/K^T built as [128, T] "m-tiles" (2 heads each) via
  out = wqk_mtile.T @ xt  with RoPE applied by
  y = raw*CC + rowswap(raw)*SS  (rowswap via SB->SB DMA partition permute).
  Scores are computed transposed: S^T[k, q] chunks [128, 512], exp on ACT,
  causal mask via affine_select, then Y'^T = [V|1]^T-chain accumulation
  giving both Y^T (rows 0-63) and softmax denominators (row 64).
"""
import sys
sys.path.insert(0, "/opt/trn_rl_repo")

import numpy as np

ROPE_BASE = 10000.0
NCORES = 8

_CACHE = {}


def _rope_tables(T):
    inv_freq = 1.0 / (ROPE_BASE ** (np.arange(0, 64, 2, dtype=np.float64) / 64))
    t = np.arange(T, dtype=np.float64)
    fr = np.outer(t, inv_freq)            # [T, 32]
    cosT = np.cos(fr).T.astype(np.float32)   # [32, T]
    sinT = np.sin(fr).T.astype(np.float32)
    CC = np.tile(cosT, (4, 1))            # [128, T]
    SS = np.concatenate([sinT, -sinT, sinT, -sinT], axis=0)  # [128, T]
    return CC, SS


def build_nc(C, T, HPC, debug=False, loop_n=1):
    """Build the per-core Bass program. C: contraction dim, T: seq len,
    HPC: heads per core (even)."""
    import concourse.bass as bass
    import concourse.tile as tile
    from concourse import bacc, mybir

    F32 = mybir.dt.float32
    F32R = mybir.dt.float32r
    Act = mybir.ActivationFunctionType

    KT = C // 128          # contraction k-tiles
    NP = HPC // 2          # head pairs
    MT = 2 * NP            # qk m-tiles (Q tiles then K tiles)
    TT = T // 128          # 128-row t-tiles (= key chunks)
    QC = T // 512          # query chunks of 512
    VC = 64 * HPC          # v columns
    PC = VC // 128         # projection contraction k-tiles (= NP)

    nc = bacc.Bacc("TRN2", target_bir_lowering=False, debug=False)

    xt_d = nc.dram_tensor("xt", [C, T], F32R, kind="ExternalInput")
    wqk_d = nc.dram_tensor("wqk", [C, 2 * VC], F32R, kind="ExternalInput")
    bqk_d = nc.dram_tensor("bqk", [2 * VC], F32, kind="ExternalInput")
    wv_d = nc.dram_tensor("wv", [C, VC], F32R, kind="ExternalInput")
    wp_d = nc.dram_tensor("wp", [VC, C], F32R, kind="ExternalInput")
    out_d = nc.dram_tensor("out", [T, C], F32, kind="ExternalOutput")
    if debug:
        KTl = C // 128; NPl = HPC // 2; MTl = HPC; TTl = T // 128; QCl = T // 512
        dbg_vp = nc.dram_tensor("dbg_vp", [128, TTl * HPC * 65], F32, kind="ExternalOutput")
        dbg_yt = nc.dram_tensor("dbg_yt", [NPl, 128, T], F32, kind="ExternalOutput")
        dbg_d = nc.dram_tensor("dbg_d", [NPl * QCl, 2, 512], F32, kind="ExternalOutput")
        dbg_bc = nc.dram_tensor("dbg_bc", [NPl * QCl, 2, 64, 512], F32, kind="ExternalOutput")

    rsc_d = nc.dram_tensor("rscratch", [HPC // 2, T // 512, 2, 512], F32R)

    CCh, SSh = _rope_tables(T)
    cc_d = nc.inline_tensor(CCh, name="rope_cc")
    ss_d = nc.inline_tensor(SSh, name="rope_ss")

    import contextlib

    @contextlib.contextmanager
    def _maybe_loop(tc):
        if loop_n > 1:
            with tc.For_i(0, loop_n, 1):
                yield
        else:
            yield

    with nc.allow_low_precision(reason="fp32r matmul pipeline"), \
         tile.TileContext(nc) as tc:
        with contextlib.ExitStack() as octx, _maybe_loop(tc), \
             contextlib.ExitStack() as ctx:
            # ---- long-lived pools -------------------------------------
            big = ctx.enter_context(tc.tile_pool(name="big", bufs=1))
            qk_pool = ctx.enter_context(tc.tile_pool(name="qks", bufs=2))
            vp_pool = ctx.enter_context(tc.tile_pool(name="vp", bufs=1))
            misc = ctx.enter_context(tc.tile_pool(name="misc", bufs=1))

            # xt and Yt share one slot (same tag, sequential lifetimes)
            xt_sb = big.tile([128, KT, T], F32R, tag="bigshare")

            vp_sb = vp_pool.tile([128, TT, HPC, 65], F32R)
            bqk_sb = misc.tile([128, MT], F32)
            nc.vector.tensor_copy(
                vp_sb[:, :, :, 64:65],
                nc.const_aps.tensor(1.0, (128, TT, HPC, 1)))

            nc.sync.dma_start(
                bqk_sb[:],
                bqk_d.ap().rearrange("(m p) -> p m", p=128))

            # ---- stage B0: weights + V projection ---------------------
            QKW = 512   # QK rope chunk width
            wqk_pool = ctx.enter_context(tc.tile_pool(name="wqk", bufs=1))
            ccss_pool = ctx.enter_context(tc.tile_pool(name="ccss", bufs=1))
            raw_pool = ctx.enter_context(tc.tile_pool(name="raw", bufs=2))
            swp_pool = ctx.enter_context(tc.tile_pool(name="swp", bufs=2))

            wqk_sb = wqk_pool.tile([128, KT, 2 * VC], F32R)
            wv_pool = ctx.enter_context(tc.tile_pool(name="wv", bufs=1))
            wv_sb = wv_pool.tile([128, KT, VC], F32R)
            cc_sb = ccss_pool.tile([128, T], F32)
            ss_sb = ccss_pool.tile([128, T], F32)
            for k in range(KT):
                nc.sync.dma_start(wqk_sb[:, k, :],
                                  wqk_d.ap()[k * 128:(k + 1) * 128, :])
                nc.scalar.dma_start(xt_sb[:, k, :],
                                    xt_d.ap()[k * 128:(k + 1) * 128, :])
            nc.scalar.dma_start(cc_sb, cc_d.ap())
            nc.scalar.dma_start(ss_sb, ss_d.ap())
            for k in range(KT):
                nc.sync.dma_start(wv_sb[:, k, :],
                                  wv_d.ap()[k * 128:(k + 1) * 128, :])

            # ---- stage BC: per-pair QK+RoPE then attention ------------
            yt_sb = big.tile([128, NP, T], F32R, tag="yt")
            with tc.tile_pool(name="mmps", bufs=2, space="PSUM") as mmps, \
                 tc.tile_pool(name="yps", bufs=2, space="PSUM") as yps, \
                 tc.tile_pool(name="pt", bufs=3) as pt_pool, \
                 tc.tile_pool(name="ra", bufs=1) as ra_pool, \
                 tc.tile_pool(name="tb", bufs=2) as tb_pool, \
                 tc.tile_pool(name="bcs", bufs=2) as bcs_pool:
                for j in range(NP):
                    # QK + RoPE for this pair's two m-tiles
                    qt = qk_pool.tile([128, T], F32R, tag="qt", name=f"qt{j}")
                    kt = qk_pool.tile([128, T], F32R, tag="kt", name=f"kt{j}")
                    for dst, m in ((qt, j), (kt, NP + j)):
                        for ch in range(T // QKW):
                            psum = mmps.tile([128, 1024], F32, tag="mm",
                                             name="qkpsum")
                            for half in range(QKW // 512):
                                hs = slice(half * 512, (half + 1) * 512)
                                xs = slice(ch * QKW + half * 512,
                                           ch * QKW + (half + 1) * 512)
                                for k in range(KT):
                                    nc.tensor.matmul(
                                        psum[:, hs],
                                        wqk_sb[:, k, m * 128:(m + 1) * 128],
                                        xt_sb[:, k, xs],
                                        start=(k == 0), stop=(k == KT - 1))
                            raw = raw_pool.tile([128, QKW], F32)
                            nc.vector.tensor_scalar_add(
                                raw, psum[:, 0:QKW], bqk_sb[:, m:m + 1])
                            swp = swp_pool.tile([128, QKW], F32)
                            for h in (0, 64):
                                nc.sync.dma_start(swp[h:h + 32, :],
                                                  raw[h + 32:h + 64, :])
                                nc.sync.dma_start(swp[h + 32:h + 64, :],
                                                  raw[h:h + 32, :])
                            cs2 = slice(ch * QKW, (ch + 1) * QKW)
                            nc.vector.tensor_mul(raw, raw, cc_sb[:, cs2])
                            nc.vector.tensor_mul(swp, swp, ss_sb[:, cs2])
                            nc.vector.tensor_add(dst[:, cs2], raw, swp)

                    if j == 0:
                        # V projection after pair-0 QK: overlaps attention
                        for tt in range(TT):
                            vpsum = mmps.tile([128, 1024], F32, tag="mm",
                                              name="vpsum")
                            for k in range(KT):
                                nc.tensor.matmul(
                                    vpsum[:, 0:VC],
                                    xt_sb[:, k, tt * 128:(tt + 1) * 128],
                                    wv_sb[:, k, :],
                                    start=(k == 0), stop=(k == KT - 1))
                            nc.vector.tensor_copy(
                                vp_sb[:, tt, :, 0:64],
                                vpsum[:, 0:VC].rearrange("p (h d) -> p h d",
                                                         h=HPC))

                    # attention for pair j
                    for qc in range(QC):
                        nkc = 4 * (qc + 1)
                        ya = yps.tile([65, 512], F32, tag="ya", name="ya")
                        yb = yps.tile([65, 512], F32, tag="yb", name="yb")
                        for kc in range(nkc):
                            qs = slice(qc * 512, (qc + 1) * 512)
                            ks = slice(kc * 128, (kc + 1) * 128)
                            spair = mmps.tile([128, 1024], F32, tag="mm",
                                              name="spair")
                            nc.tensor.matmul(spair[:, 0:512], kt[0:64, ks],
                                             qt[0:64, qs], start=True, stop=True)
                            nc.tensor.matmul(spair[:, 512:1024], kt[64:128, ks],
                                             qt[64:128, qs], start=True, stop=True)
                            pp = pt_pool.tile([128, 1024], F32R, tag="pp")
                            nc.scalar.activation(pp, spair, Act.Exp, scale=0.125)
                            if kc >= 4 * qc:  # diagonal: mask k > q -> 0
                                nc.gpsimd.affine_select(
                                    out=pp, in_=pp,
                                    compare_op=mybir.AluOpType.is_ge,
                                    fill=0.0,
                                    base=qc * 512 - kc * 128,
                                    channel_multiplier=-1,
                                    pattern=[[0, 2], [1, 512]])
                            nc.tensor.matmul(ya, vp_sb[:, kc, 2 * j, :],
                                             pp[:, 0:512],
                                             start=(kc == 0), stop=(kc == nkc - 1))
                            nc.tensor.matmul(yb, vp_sb[:, kc, 2 * j + 1, :],
                                             pp[:, 512:1024],
                                             start=(kc == 0), stop=(kc == nkc - 1))
                        qs = slice(qc * 512, (qc + 1) * 512)
                        ra = ra_pool.tile([65, 512], F32R, tag="ra")
                        rb = ra_pool.tile([65, 512], F32R, tag="rb")
                        nc.vector.reciprocal(ra[64:65, :], ya[64:65, :])
                        nc.vector.reciprocal(rb[64:65, :], yb[64:65, :])
                        bca = bcs_pool.tile([64, 512], F32R, tag="bca")
                        bcb = bcs_pool.tile([64, 512], F32R, tag="bcb")
                        nc.sync.dma_start(rsc_d.ap()[j, qc, 0], ra[64:65, :])
                        nc.sync.dma_start(rsc_d.ap()[j, qc, 1], rb[64:65, :])
                        nc.sync.dma_start(
                            bca, rsc_d.ap()[j, qc, 0:1, :].to_broadcast((64, 512)))
                        nc.sync.dma_start(
                            bcb, rsc_d.ap()[j, qc, 1:2, :].to_broadcast((64, 512)))
                        nc.vector.tensor_mul(yt_sb[0:64, j, qs], ya[0:64, :], bca)
                        tb = tb_pool.tile([64, 512], F32R)
                        nc.vector.tensor_mul(tb, yb[0:64, :], bcb)
                        nc.sync.dma_start(yt_sb[64:128, j, qs], tb)
                        if debug:
                            nc.sync.dma_start(dbg_d.ap()[j * QC + qc, 0], ra[64:65, :].bitcast(F32))
                            nc.sync.dma_start(dbg_d.ap()[j * QC + qc, 1], rb[64:65, :].bitcast(F32))
                            nc.sync.dma_start(dbg_bc.ap()[j * QC + qc, 0], bca[:].bitcast(F32))
                            nc.sync.dma_start(dbg_bc.ap()[j * QC + qc, 1], bcb[:].bitcast(F32))

            if debug:
                nc.sync.dma_start(
                    dbg_vp.ap(),
                    vp_sb[:].rearrange("p a b c -> p (a b c)").bitcast(F32))
                for j in range(NP):
                    nc.sync.dma_start(dbg_yt.ap()[j], yt_sb[:, j, :].bitcast(F32))

            # ---- stage D: output projection ---------------------------
            with tc.tile_pool(name="wp", bufs=1) as wp_pool, \
                 tc.tile_pool(name="osb", bufs=3) as osb_pool, \
                 tc.tile_pool(name="pps", bufs=3, space="PSUM") as pps:
                wp_sb = wp_pool.tile([128, PC, C], F32R)
                for k in range(PC):
                    nc.sync.dma_start(wp_sb[:, k, :],
                                      wp_d.ap()[k * 128:(k + 1) * 128, :])
                ccw = 384 if C % 384 == 0 else C  # proj column chunk width
                ncc = C // ccw
                for tt in range(TT):
                    osb = osb_pool.tile([128, C], F32)
                    for cc in range(ncc):
                        cs = slice(cc * ccw, (cc + 1) * ccw)
                        psum = pps.tile([128, ccw], F32)
                        for k in range(PC):
                            nc.tensor.matmul(
                                psum, yt_sb[:, k, tt * 128:(tt + 1) * 128],
                                wp_sb[:, k, cs],
                                start=(k == 0), stop=(k == PC - 1))
                        nc.vector.tensor_copy(osb[:, cs], psum)
                    nc.sync.dma_start(out_d.ap()[tt * 128:(tt + 1) * 128, :], osb)

    nc.compile()
    return nc


def _run(nc, in_maps):
    from concourse.bass_utils import run_bass_kernel_spmd
    return run_bass_kernel_spmd(nc, in_maps, core_ids=list(range(len(in_maps))))


def kernel(**inputs):
    x = np.ascontiguousarray(np.asarray(inputs["x"], dtype=np.float32))
    W = np.asarray(inputs["W_attn"], dtype=np.float32)
    b = np.asarray(inputs["b_attn"], dtype=np.float32)
    Wp = np.asarray(inputs["W_proj"], dtype=np.float32)
    bp = np.asarray(inputs["b_proj"], dtype=np.float32)
    B, T, C = x.shape

    if "nc" not in _CACHE:
        _CACHE["nc"] = build_nc(C, T, 6)
    nc = _CACHE["nc"]

    in_maps = []
    for c in range(NCORES):
        bb, g = divmod(c, 2)
        s = 384 * g
        in_maps.append({
            "xt": np.ascontiguousarray(x[bb].T),
            "wqk": np.ascontiguousarray(
                np.concatenate([W[:, s:s + 384], W[:, 768 + s:768 + s + 384]],
                               axis=1)),
            "bqk": np.ascontiguousarray(
                np.concatenate([b[s:s + 384], b[768 + s:768 + s + 384]])),
            "wv": np.ascontiguousarray(W[:, 1536 + s:1536 + s + 384]),
            "wp": np.ascontiguousarray(Wp[s:s + 384, :]),
        })

    res = _run(nc, in_maps).results
    extra = (bp + b[1536:2304] @ Wp).astype(np.float32)  # [C]
    out = np.empty((B, T, C), dtype=np.float32)
    for bb in range(B):
        out[bb] = res[2 * bb]["out"] + res[2 * bb + 1]["out"] + extra
    return out
